# revision 1
# baseline (speedup 1.0000x reference)
"""MultiHeadInfiniAttention Trainium2 kernel (8 NeuronCores).

Problem: B=2, T=4096, D=1024, H=8 heads x 128 dh, SEG_LEN=512 (8 segments).
Per (b,h): segment-recurrent memory (M [128,129 incl z]) + local causal
softmax attention, gated combine.

Sharding: 16 (b,h) pairs over 8 cores -> core c handles b=c//4 and heads
{2*(c%4), 2*(c%4)+1}.  Host passes per-core inputs: xT=x[b].T (layout-only),
weight column slices, bias/gate columns, small constant matrices.

On-device dataflow per (segment s, head h):
  - projections qT/kT/vT [dh,512] = sum_c W[c]^T @ xT[c, seg]  (float32r MMs)
  - sq/sk = elu+1 in bf16; memory read a_mem/retr via lhsT=sqT/skT against
    M||z (evacuated per-pair immediately -> no psum WAR cycles)
  - scores/softmax in [t,m] layout (causal block-skipping; diagonal mask
    added via a rank-128 matmul; ACT exp with fused row-sum denominator);
    P transposed through PE for a_dot
  - delta-rule update M||z += sk^T @ (v - retr/(sk.z) || 1)
"""

import os
import sys

sys.path.insert(0, os.path.dirname(os.path.abspath(__file__)))

import numpy as np
import ml_dtypes

import concourse.bass as bass
import concourse.mybir as mybir
import concourse.tile as tile
from concourse import bass_utils
from concourse.bass import ts


def split_multi_waits(nc, max_waits: int = 1) -> int:
    """This container's walrus build only supports ONE sync wait per
    instruction.  Tile emits multi-wait instructions; split the extras onto
    same-engine NOP carriers inserted right before each instruction."""
    n_split = 0
    for func in nc.m.functions:
        for bb in func.blocks:
            insts = bb.instructions
            new_list = []
            changed = False
            for inst in insts:
                si = inst.sync_info
                if si is not None and si.on_wait and len(si.on_wait) > max_waits:
                    waits = list(si.on_wait)
                    for w in waits[max_waits:]:
                        nop = mybir.InstNoOp(name=f"WSPLIT-{nc.next_id()}")
                        nop.engine = inst.engine
                        nop.sync_info = mybir.SyncInfo(on_wait=[w], on_update=[])
                        new_list.append(nop)
                        n_split += 1
                    inst.sync_info = mybir.SyncInfo(
                        on_wait=waits[:max_waits],
                        on_update=list(si.on_update or []),
                    )
                    changed = True
                new_list.append(inst)
            if changed:
                bb.instructions = new_list
    return n_split


F32 = mybir.dt.float32
F32R = mybir.dt.float32r
BF16 = mybir.dt.bfloat16
AF = mybir.ActivationFunctionType
ALU = mybir.AluOpType

B, T, D = 2, 4096, 1024
H, DH, SEG = 8, 128, 512
S = T // SEG          # 8 segments
NCH = D // 128        # 8 contraction chunks
EPS = 1e-6
INV_SQRT_D = 1.0 / float(np.sqrt(DH))
MASK_NEG = -1.0e9

LAST_RESULTS = None  # BassKernelResults of the last run (for test.py)


def _build_program():
    nc = bass.Bass("TRN2", target_bir_lowering=False, debug=False)

    xT = nc.dram_tensor("xT", (D, T), F32R, kind="ExternalInput")
    wq = nc.dram_tensor("wq", (D, 2 * DH), F32R, kind="ExternalInput")
    wk = nc.dram_tensor("wk", (D, 2 * DH), F32R, kind="ExternalInput")
    wv = nc.dram_tensor("wv", (D, 2 * DH), F32R, kind="ExternalInput")
    biases = nc.dram_tensor("biases", (128, 6), F32, kind="ExternalInput")
    gates = nc.dram_tensor("gates", (128, 4), F32, kind="ExternalInput")
    ident_d = nc.dram_tensor("ident", (128, 128), BF16, kind="ExternalInput")
    maskl_d = nc.dram_tensor("maskl", (128, 128), BF16, kind="ExternalInput")
    maskr_d = nc.dram_tensor("maskr", (128, 128), BF16, kind="ExternalInput")
    y = nc.dram_tensor("out", (T, 2 * DH), F32, kind="ExternalOutput")

    with tile.TileContext(nc) as tc:
        _emit(nc, tc, xT, wq, wk, wv, biases, gates, ident_d, maskl_d, maskr_d, y)

    split_multi_waits(nc)
    return nc


def _emit(nc, tc, xT, wq, wk, wv, biases, gates, ident_d, maskl_d, maskr_d, y):
    from contextlib import ExitStack

    ctx = ExitStack()
    with ctx:
        singles = ctx.enter_context(tc.tile_pool(name="singles", bufs=1))
        state = ctx.enter_context(tc.tile_pool(name="state", bufs=2))
        xpool = ctx.enter_context(tc.tile_pool(name="xts", bufs=4))
        work = ctx.enter_context(tc.tile_pool(name="work", bufs=4))
        small = ctx.enter_context(tc.tile_pool(name="small", bufs=8))
        outp = ctx.enter_context(tc.tile_pool(name="outp", bufs=4))
        # PSUM pools -- exactly 8 banks
        proj_ps = ctx.enter_context(tc.tile_pool(name="proj_ps", bufs=2, space="PSUM"))
        trp_ps = proj_ps  # transposes share the projection psum slots
        sc_ps_p = ctx.enter_context(tc.tile_pool(name="sc_ps", bufs=2, space="PSUM"))
        adot_ps_p = ctx.enter_context(tc.tile_pool(name="adot_ps", bufs=2, space="PSUM"))
        mem_ps_p = ctx.enter_context(tc.tile_pool(name="mem_ps", bufs=2, space="PSUM"))

        # ---- constants ----
        # Small consts + weights go on the ACT HWDGE queue; xts slabs and
        # output stores use the SP queue, so startup overlaps.  Weights are
        # split per contraction chunk so the first projection matmuls can
        # start after ~1 chunk of wq instead of all 3 weight matrices.
        # load order on the sync queue: wq -> segment-0 x slab (split per
        # chunk for incremental matmul start) -> wk/wv -> remaining slabs
        # (one large DMA each; per-dma_start issue overhead is ~0.6us).
        w_sb = {}
        w_views = {}
        for name, dram in (("wq", wq), ("wk", wk), ("wv", wv)):
            w_sb[name] = singles.tile(
                [128, NCH, 2 * DH], F32R, tag=f"w_{name}", name=f"w_{name}"
            )
            w_views[name] = dram.ap().rearrange("(c p) n -> p c n", p=128)

        # ---- persistent per-head state ----
        # mzb double-buffered per head: segment s reads buf[(s-1)%2] (old M)
        # while the update writes buf[s%2], so the chain write never waits
        # on this segment's readers.
        mz_f32, mz_bf = [], []
        for hi in range(2):
            mzf = state.tile([128, DH + 1], F32, tag="mz_f32")
            bufs2 = [
                state.tile([128, DH + 1], BF16, tag="mz_bf", bufs=4,
                           name=f"mzb_{hi}_{k}")
                for k in range(2)
            ]
            mz_f32.append(mzf)
            mz_bf.append(bufs2)

        yv = y.ap().rearrange(
            "(s tile p) (h e) -> s p tile h e", p=128, tile=4, h=2
        )
        # x^T slab view: slab[p, c, f] = xT[c*128 + p, s*512 + f]
        xv = xT.ap().rearrange("(c p) t -> p c t", p=128)

        def load_slab(s, split):
            slab = xpool.tile([128, NCH, SEG], F32R, tag="slab", name=f"slab{s}")
            if split:
                for c in range(NCH):
                    nc.sync.dma_start(out=slab[:, c, :], in_=xv[:, c, ts(s, SEG)])
            else:
                nc.sync.dma_start(out=slab[:], in_=xv[:, :, ts(s, SEG)])
            return slab

        # interleave all three weights' chunk-pairs with slab-0 chunks so
        # q, k and v projections of segment 0 all unblock incrementally
        slab0 = xpool.tile([128, NCH, SEG], F32R, tag="slab", name="slab0")
        for g in range(4):
            for name in ("wq", "wk", "wv"):
                nc.sync.dma_start(
                    out=w_sb[name][:, 2 * g : 2 * g + 2, :],
                    in_=w_views[name][:, 2 * g : 2 * g + 2, :],
                )
            for c in (2 * g, 2 * g + 1):
                nc.sync.dma_start(out=slab0[:, c, :], in_=xv[:, c, ts(0, SEG)])
        bias_sb = singles.tile([128, 6], F32, tag="bias")
        nc.scalar.dma_start(out=bias_sb[:], in_=biases.ap())
        ident = singles.tile([128, 128], BF16, tag="ident")
        nc.scalar.dma_start(out=ident[:], in_=ident_d.ap())
        gate_sb = singles.tile([128, 4], F32, tag="gate")
        nc.scalar.dma_start(out=gate_sb[:], in_=gates.ap())
        maskl = singles.tile([128, 128], BF16, tag="maskl")
        nc.scalar.dma_start(out=maskl[:], in_=maskl_d.ap())
        maskr = singles.tile([128, 128], BF16, tag="maskr")
        nc.scalar.dma_start(out=maskr[:], in_=maskr_d.ap())

        # Software-pipelined emission: the "produce" phase (projections, elu,
        # layout transposes) of segment s+1 is emitted before the serial
        # "scan" phase of segment s, so the scheduler can fill the scan's
        # dependency stalls with projection matmuls.
        def produce(s, slab):
            xts = [slab[:, c, :] for c in range(NCH)]
            return [
                _produce_phase(
                    nc, s, hi, xts, w_sb, bias_sb, ident,
                    work, proj_ps, trp_ps,
                )
                for hi in range(2)
            ]

        for s in range(S):
            slab = slab0 if s == 0 else load_slab(s, split=(s == 1))
            pr = produce(s, slab)
            # layout [p, tile, head, e] so the store DMA collapses to 2D
            a2_sb = outp.tile([128, 4, 2, 128], F32, tag="a2_sb", name=f"a2_{s}")
            for hi in range(2):
                _scan_phase(
                    nc, tc, s, hi, pr[hi], gate_sb, ident, maskl, maskr,
                    mz_f32[hi], mz_bf[hi][(s - 1) % 2], mz_bf[hi][s % 2],
                    work, small,
                    sc_ps_p, trp_ps, adot_ps_p, mem_ps_p,
                    a2_sb[:, :, hi, :],
                )
                if s == S - 1:
                    nc.scalar.dma_start(
                        out=yv[s, :, :, hi], in_=a2_sb[:, :, hi, :]
                    )
            if s < S - 1:
                nc.scalar.dma_start(out=yv[s], in_=a2_sb[:])


def _produce_phase(
    nc, s, hi, xts, w_sb, bias_sb, ident, work, proj_ps, trp_ps,
):
    # ---------- projections: qT/kT/vT [dh, 512] (float32r) ----------
    def project(wname):
        ps = proj_ps.tile([128, SEG], F32, tag="proj", name=f"proj_{wname}_{s}_{hi}")
        w = w_sb[wname]
        for c in range(NCH):
            nc.tensor.matmul(
                ps[:], w[:, c, ts(hi, DH)], xts[c],
                start=(c == 0), stop=(c == NCH - 1),
            )
        return ps

    qt_ps = project("wq")
    q_bf = work.tile([128, SEG], BF16, tag="q_bf", bufs=5)
    nc.scalar.activation(q_bf[:], qt_ps[:], AF.Identity, bias=bias_sb[:, 0 + hi : 1 + hi])

    kt_ps = project("wk")
    k_bf = work.tile([128, SEG], BF16, tag="k_bf", bufs=5)
    nc.scalar.activation(k_bf[:], kt_ps[:], AF.Identity, bias=bias_sb[:, 2 + hi : 3 + hi])

    vt_ps = project("wv")
    vt_bf = work.tile([128, SEG], BF16, tag="vt_bf", bufs=5)
    nc.scalar.activation(vt_bf[:], vt_ps[:], AF.Identity, bias=bias_sb[:, 4 + hi : 5 + hi])

    # ---------- elu(x)+1 = exp(min(x,0)) + relu(x), bf16 ----------
    def elu1(src_bf, tag):
        mn = work.tile([128, SEG], BF16, tag=f"mn_{tag}", bufs=3)
        nc.vector.tensor_scalar_min(mn[:], src_bf[:], 0.0)
        ex = work.tile([128, SEG], BF16, tag=f"ex_{tag}", bufs=3)
        nc.scalar.activation(ex[:], mn[:], AF.Exp)
        out = work.tile([128, SEG], BF16, tag=f"s_{tag}", bufs=5)
        nc.vector.scalar_tensor_tensor(
            out=out[:], in0=src_bf[:], scalar=0.0, in1=ex[:],
            op0=ALU.max, op1=ALU.add,
        )
        return out

    sq_bf = elu1(q_bf, "q") if s > 0 else None       # sqT (amem only)
    sk_bf = elu1(k_bf, "k") if s < S - 1 else None   # skT [dh, t]

    # ---------- natural-layout v and sk via PE transpose ----------
    def to_natural(src_bf, tag, engine, ones_col=False):
        ps = trp_ps.tile([128, 4, 128], BF16, tag="proj", name=f"trp_{tag}_{s}_{hi}")
        for i in range(4):
            nc.tensor.transpose(ps[:, i, :], src_bf[:, ts(i, 128)], ident[:])
        cols = DH + 1 if ones_col else DH
        nat = work.tile([128, 4, cols], BF16, tag=f"nat_{tag}", bufs=5,
                        name=f"nat_{tag}_{s}_{hi}")
        if ones_col:
            nc.gpsimd.memset(nat[:, :, DH : DH + 1], 1.0)
        if engine == "act":
            nc.scalar.copy(nat[:, :, :DH], ps[:])
        else:
            nc.vector.tensor_copy(nat[:, :, :DH], ps[:])
        return nat

    # v_ones [m, 4, dh+1]: natural-layout v with a ones column, so the
    # a_dot matmul accumulates the softmax denominator in column dh.
    v_ones = to_natural(vt_bf, "v", "act", ones_col=True)
    sk_nat = to_natural(sk_bf, "sk", "vec") if s < S - 1 else None

    return dict(q_bf=q_bf, k_bf=k_bf, sq_bf=sq_bf, sk_bf=sk_bf,
                v_ones=v_ones, sk_nat=sk_nat)


def _scan_phase(
    nc, tc, s, hi, pr, gate_sb, ident, maskl, maskr,
    mzf, mzb_prev, mzb_new, work, small,
    sc_ps_p, trp_ps, adot_ps_p, mem_ps_p, a_sb,
):
    q_bf, k_bf = pr["q_bf"], pr["k_bf"]
    sq_bf, sk_bf = pr["sq_bf"], pr["sk_bf"]
    v_ones, sk_nat = pr["v_ones"], pr["sk_nat"]

    # ---------- memory state pipeline ----------
    # M update is decomposed as  M||z += sk^T @ (v||1)  +  sk^T @ (retr*(-rkn))
    # so only the second term sits on the cross-segment critical chain.
    if s < S - 1:
        uc_ps = mem_ps_p.tile([128, DH + 1], F32, tag="mem", name=f"uc_{s}_{hi}")
        for j in range(4):
            nc.tensor.matmul(
                uc_ps[:], sk_nat[:, j, :], v_ones[:, j, :],
                start=(j == 0), stop=(s == 0 and j == 3),
                skip_group_check=True,
            )
    # retr side (the chain): retr = sk @ M; retr_n = retr * (-rkn)
    amem_cat = None
    if 0 < s < S - 1:
        retr_n = work.tile([128, 4, 128], BF16, tag="retr_n")
        for pair in range(2):
            rps = mem_ps_p.tile([128, 2, DH + 1], F32, tag="mem",
                                name=f"retr_{s}_{hi}_{pair}")
            for i2 in range(2):
                nc.tensor.matmul(
                    rps[:, i2, :], sk_bf[:, ts(pair * 2 + i2, 128)], mzb_prev[:],
                    start=(i2 == 0), stop=(i2 == 1), skip_group_check=True,
                )
            rkn = small.tile([128, 2], F32, tag="rkn", name=f"rkn_{s}_{hi}_{pair}")
            nc.vector.tensor_scalar(
                rkn[:], rps[:, :, DH], EPS, -1.0, ALU.add, ALU.mult
            )
            nc.vector.reciprocal(rkn[:], rkn[:])
            rkn_bc = bass.AP(
                tensor=rkn.tensor, offset=rkn.offset,
                ap=[rkn.ap[0], rkn.ap[1], [0, 128]],
            )
            nc.vector.tensor_mul(
                retr_n[:, 2 * pair : 2 * pair + 2, :], rps[:, :, :DH], rkn_bc
            )
        if s < S - 1:
            for j in range(4):
                nc.tensor.matmul(
                    uc_ps[:, :DH], sk_nat[:, j, :], retr_n[:, j, :],
                    start=False, stop=(j == 3), skip_group_check=True,
                )
    if s < S - 1:
        if s == 0:
            nc.vector.tensor_copy(mzb_new[:], uc_ps[:])
            nc.vector.tensor_copy(mzf[:], uc_ps[:])
        else:
            nc.vector.scalar_tensor_tensor(
                out=mzb_new[:], in0=uc_ps[:], scalar=1.0, in1=mzf[:],
                op0=ALU.mult, op1=ALU.add,
            )
            if s < S - 2:  # mzf(S-2) has no reader (S-1 skips the update)
                nc.vector.tensor_add(mzf[:], mzf[:], uc_ps[:])

    # a_mem side (off-chain): amem_cat = gate * (sq @ M) / (sq.z + eps)
    if s > 0:
        amem_cat = work.tile([128, 4, 128], F32, tag="amem_cat")
        for pair in range(2):
            aps = mem_ps_p.tile([128, 2, DH + 1], F32, tag="mem",
                                name=f"amem_{s}_{hi}_{pair}")
            for i2 in range(2):
                nc.tensor.matmul(
                    aps[:, i2, :], sq_bf[:, ts(pair * 2 + i2, 128)], mzb_prev[:],
                    start=(i2 == 0), stop=(i2 == 1), skip_group_check=True,
                )
            rg = small.tile([128, 2], F32, tag="rg", name=f"rg_{s}_{hi}_{pair}")
            nc.vector.tensor_scalar_add(rg[:], aps[:, :, DH], EPS)
            nc.vector.reciprocal(rg[:], rg[:])
            nc.vector.tensor_scalar_mul(rg[:], rg[:], gate_sb[:, 2 * hi : 2 * hi + 1])
            if s >= S - 2:
                # tail is ACT-heavy: do the scale on DVE in one bcast op
                rg_bc = bass.AP(
                    tensor=rg.tensor, offset=rg.offset,
                    ap=[rg.ap[0], rg.ap[1], [0, 128]],
                )
                nc.vector.tensor_mul(
                    amem_cat[:, 2 * pair : 2 * pair + 2, :],
                    aps[:, :, :DH], rg_bc,
                )
            else:
                for i2 in range(2):
                    nc.scalar.activation(
                        amem_cat[:, pair * 2 + i2, :], aps[:, i2, :DH],
                        AF.Identity, scale=rg[:, i2 : i2 + 1],
                    )

    # ---------- local causal attention (transposed-scores formulation) ----
    # scoresT_j [m-chunk j, t >= j*128] = k_j^T q; diagonal mask added via
    # (maskr^T maskl)[m,t] = MASK_NEG iff m > t; ACT exp writes P^T directly
    # (no PE transposes); a_dot accumulates against v||1 so column dh holds
    # the softmax denominator.
    adot_pair = []
    for pair in range(2):
        adot_pair.append(
            adot_ps_p.tile([128, 2, DH + 1], F32, tag="adot",
                           name=f"adot_{s}_{hi}_{pair}")
        )
    for j in range(4):
        t_cols = (4 - j) * 128
        sc = sc_ps_p.tile([128, SEG], F32, tag="scores")
        nc.tensor.matmul(
            sc[:, :t_cols], k_bf[:, ts(j, 128)], q_bf[:, j * 128 :],
            start=True, stop=False, skip_group_check=True,
        )
        nc.tensor.matmul(
            sc[:, :128], maskr[:], maskl[:],
            start=False, stop=True, skip_group_check=True,
        )
        ptj = work.tile([128, t_cols], BF16, tag=f"pt{j}", bufs=2,
                        name=f"pt{j}_{s}_{hi}")
        nc.scalar.activation(ptj[:], sc[:, :t_cols], AF.Exp, scale=INV_SQRT_D)
        for i in range(j, 4):
            pair, i2 = divmod(i, 2)
            # start=True clears has_written BANK-wide: only the first
            # region per bank may carry it; the second region's first
            # write stores via the already-cleared bits.
            nc.tensor.matmul(
                adot_pair[pair][:, i2, :], ptj[:, ts(i - j, 128)],
                v_ones[:, j, :],
                start=(j == 0 and i2 == 0), stop=(j == i),
                skip_group_check=True,
            )

    # ---------- combine ----------
    for pair in range(2):
        rdot = small.tile([128, 2], F32, tag="rdot", name=f"rdot_{s}_{hi}_{pair}")
        nc.vector.reciprocal(rdot[:], adot_pair[pair][:, :, DH])
        nc.vector.tensor_scalar_mul(
            rdot[:], rdot[:], gate_sb[:, 2 * hi + 1 : 2 * hi + 2]
        )
        rdot_bc = bass.AP(
            tensor=rdot.tensor, offset=rdot.offset,
            ap=[rdot.ap[0], rdot.ap[1], [0, 128]],
        )
        a_slice = a_sb[:, 2 * pair : 2 * pair + 2, :]
        if s > 0:
            tmp = work.tile([128, 2, 128], F32, tag="a_tmp",
                            name=f"a_tmp_{s}_{hi}_{pair}")
            nc.vector.tensor_mul(tmp[:], adot_pair[pair][:, :, :DH], rdot_bc)
            nc.vector.tensor_add(
                a_slice, tmp[:], amem_cat[:, 2 * pair : 2 * pair + 2, :]
            )
        else:
            nc.vector.tensor_mul(a_slice, adot_pair[pair][:, :, :DH], rdot_bc)


_NC_CACHE = None


def _get_nc():
    global _NC_CACHE
    if _NC_CACHE is None:
        _NC_CACHE = _build_program()
    return _NC_CACHE


def _host_consts():
    ident = np.eye(128, dtype=ml_dtypes.bfloat16)
    # maskl[k,t] = 1 iff k > t  ->  (maskl^T @ maskr)[t,m] = MASK_NEG iff m > t
    maskl = np.tril(np.ones((128, 128), np.float32), -1).astype(ml_dtypes.bfloat16)
    maskr = (MASK_NEG * np.eye(128, dtype=np.float32)).astype(ml_dtypes.bfloat16)
    return ident, maskl, maskr


def kernel(x, w_q, b_q, w_k, b_k, w_v, b_v, beta, _trace=False):
    global LAST_RESULTS
    x = np.asarray(x, dtype=np.float32)
    w_q = np.asarray(w_q, dtype=np.float32)
    b_q = np.asarray(b_q, dtype=np.float32)
    w_k = np.asarray(w_k, dtype=np.float32)
    b_k = np.asarray(b_k, dtype=np.float32)
    w_v = np.asarray(w_v, dtype=np.float32)
    b_v = np.asarray(b_v, dtype=np.float32)
    beta = np.asarray(beta, dtype=np.float32)

    gate = 1.0 / (1.0 + np.exp(-beta))  # sigmoid, [H]
    ident, maskl, maskr = _host_consts()

    in_maps = []
    for c in range(8):
        b = c // 4
        h0 = (c % 4) * 2
        cols = slice(h0 * DH, (h0 + 2) * DH)
        bias_cols = np.stack(
            [
                b_q[h0 * DH : (h0 + 1) * DH], b_q[(h0 + 1) * DH : (h0 + 2) * DH],
                b_k[h0 * DH : (h0 + 1) * DH], b_k[(h0 + 1) * DH : (h0 + 2) * DH],
                b_v[h0 * DH : (h0 + 1) * DH], b_v[(h0 + 1) * DH : (h0 + 2) * DH],
            ],
            axis=1,
        ).astype(np.float32)  # [128, 6]
        g0, g1 = gate[h0], gate[h0 + 1]
        gates_np = np.tile(
            np.array([g0, 1.0 - g0, g1, 1.0 - g1], np.float32), (128, 1)
        )
        in_maps.append(
            {
                "xT": np.ascontiguousarray(x[b].T),
                "wq": np.ascontiguousarray(w_q[:, cols]),
                "wk": np.ascontiguousarray(w_k[:, cols]),
                "wv": np.ascontiguousarray(w_v[:, cols]),
                "biases": np.ascontiguousarray(bias_cols),
                "gates": gates_np,
                "ident": ident,
                "maskl": maskl,
                "maskr": maskr,
            }
        )

    nc = _get_nc()
    LAST_RESULTS = bass_utils.run_bass_kernel_spmd(
        nc, in_maps, core_ids=list(range(8)), trace=_trace
    )

    out = np.empty((B, T, H * DH), np.float32)
    for c in range(8):
        b = c // 4
        h0 = (c % 4) * 2
        out[b, :, h0 * DH : (h0 + 2) * DH] = LAST_RESULTS.results[c]["out"]
    return out



# revision 10
# speedup vs baseline: 1.1903x; 1.1903x over previous
"""MultiHeadInfiniAttention Trainium2 kernel (8 NeuronCores).

Problem: B=2, T=4096, D=1024, H=8 heads x 128 dh, SEG_LEN=512 (8 segments).
Per (b,h): segment-recurrent memory (M [128,129 incl z]) + local causal
softmax attention, gated combine.

Sharding: 16 (b,h) pairs over 8 cores -> core c handles b=c//4 and heads
{2*(c%4), 2*(c%4)+1}.  Host passes per-core inputs: fp8 hi/lo splits of
x[b].T and of the weight column slices, bias/gate columns, small consts.

Projections run as scale-matched 3-term fp8 DoubleRow matmuls (4x PE rate):
  x = xh(e4m3) + xl(e5m2)           [xl at natural scale: e5m2's wide
  64*w = wh(e4m3) + wl(e5m2)         exponent range holds the residual]
  64*q = xh@wh + xl@wh + xh@wl      [all three terms share PSUM scale ->
                                     one accumulation group, no fixups]
The trailing 2^-6 rides the existing PSUM->SBUF activation copy.

v is projected directly into natural [token, dh] layout (lhsT=x chunks),
removing the PE transposes; the causal diagonal mask is applied by a Pool
(gpsimd) multiply on exp(scores) instead of a PE mask matmul.

On-device dataflow per (segment s, head h):
  - projections qT/kT [dh,512] (fp8 DoubleRow, 12 matmuls each), v_nat
    [tok,2*dh] likewise; sq/sk = elu+1 in bf16 batched across both heads
  - scoresT [m,t] (causal block-skipping); ACT exp; Pool masks the
    diagonal block; a_dot accumulates against v||1 so column dh holds the
    softmax denominator
  - memory read a_mem/retr via lhsT=sqT/skT against M||z; delta-rule
    update M||z += sk^T @ (v - retr/(sk.z) || 1)
  - combine: one scalar_tensor_tensor per 128-token chunk
    (a = a_dot*rdot + amem_cat)
"""

import os
import sys

sys.path.insert(0, os.path.dirname(os.path.abspath(__file__)))

import numpy as np
import ml_dtypes

import concourse.bass as bass
import concourse.mybir as mybir
import concourse.tile as tile
from concourse import bass_utils
from concourse.bass import ts


def split_multi_waits(nc, max_waits: int = 1) -> int:
    """This container's walrus build only supports ONE sync wait per
    instruction.  Tile emits multi-wait instructions; split the extras onto
    same-engine NOP carriers inserted right before each instruction."""
    n_split = 0
    for func in nc.m.functions:
        for bb in func.blocks:
            insts = bb.instructions
            new_list = []
            changed = False
            for inst in insts:
                si = inst.sync_info
                if si is not None and si.on_wait and len(si.on_wait) > max_waits:
                    waits = list(si.on_wait)
                    for w in waits[max_waits:]:
                        nop = mybir.InstNoOp(name=f"WSPLIT-{nc.next_id()}")
                        nop.engine = inst.engine
                        nop.sync_info = mybir.SyncInfo(on_wait=[w], on_update=[])
                        new_list.append(nop)
                        n_split += 1
                    inst.sync_info = mybir.SyncInfo(
                        on_wait=waits[:max_waits],
                        on_update=list(si.on_update or []),
                    )
                    changed = True
                new_list.append(inst)
            if changed:
                bb.instructions = new_list
    return n_split


F32 = mybir.dt.float32
BF16 = mybir.dt.bfloat16
F8H = mybir.dt.float8e4   # e4m3 (hi parts)
F8L = mybir.dt.float8e5   # e5m2 (residual parts)
AF = mybir.ActivationFunctionType
ALU = mybir.AluOpType
DR = mybir.MatmulPerfMode.DoubleRow

B, T, D = 2, 4096, 1024
H, DH, SEG = 8, 128, 512
S = T // SEG          # 8 segments
NCH = D // 128        # 8 contraction chunks (4 DoubleRow chunk-pairs)
NPAIR = NCH // 2
EPS = 1e-6
WSCALE = 64.0         # weights quantized at 64*w; 2^-6 folded into copies
INV_WS = 1.0 / WSCALE
INV_SQRT_D = 1.0 / float(np.sqrt(DH))

LAST_RESULTS = None  # BassKernelResults of the last run (for test.py)


def _build_program(vbias_zero: bool):
    nc = bass.Bass("TRN2", target_bir_lowering=False, debug=False)

    xh = nc.dram_tensor("xh", (128, NCH, T), F8H, kind="ExternalInput")
    xl = nc.dram_tensor("xl", (128, NCH, T), F8L, kind="ExternalInput")
    w_dram = {}
    for nm in ("q", "k", "v"):
        w_dram[nm] = (
            nc.dram_tensor(f"wh{nm}", (128, NCH, 2 * DH), F8H, kind="ExternalInput"),
            nc.dram_tensor(f"wl{nm}", (128, NCH, 2 * DH), F8L, kind="ExternalInput"),
        )
    biases = nc.dram_tensor("biases", (128, 6), F32, kind="ExternalInput")
    gates = nc.dram_tensor("gates", (128, 4), F32, kind="ExternalInput")
    ident_d = nc.dram_tensor("ident", (128, 128), BF16, kind="ExternalInput")
    trimask_d = nc.dram_tensor("trimask", (128, 128), BF16, kind="ExternalInput")
    vb64_d = nc.dram_tensor("vb64", (128, 2 * DH), BF16, kind="ExternalInput")
    y = nc.dram_tensor("out", (T, 2 * DH), F32, kind="ExternalOutput")

    with tile.TileContext(nc) as tc:
        _emit(nc, tc, xh, xl, w_dram, biases, gates, ident_d, trimask_d,
              vb64_d, y, vbias_zero)

    split_multi_waits(nc)
    return nc


def _emit(nc, tc, xh, xl, w_dram, biases, gates, ident_d, trimask_d,
          vb64_d, y, vbias_zero):
    from contextlib import ExitStack

    ctx = ExitStack()
    with ctx:
        singles = ctx.enter_context(tc.tile_pool(name="singles", bufs=1))
        state = ctx.enter_context(tc.tile_pool(name="state", bufs=2))
        xpool = ctx.enter_context(tc.tile_pool(name="xts", bufs=3))
        work = ctx.enter_context(tc.tile_pool(name="work", bufs=4))
        small = ctx.enter_context(tc.tile_pool(name="small", bufs=8))
        outp = ctx.enter_context(tc.tile_pool(name="outp", bufs=4))
        # PSUM pools -- exactly 8 banks
        proj_ps = ctx.enter_context(tc.tile_pool(name="proj_ps", bufs=2, space="PSUM"))
        sc_ps_p = ctx.enter_context(tc.tile_pool(name="sc_ps", bufs=2, space="PSUM"))
        adot_ps_p = ctx.enter_context(tc.tile_pool(name="adot_ps", bufs=2, space="PSUM"))
        mem_ps_p = ctx.enter_context(tc.tile_pool(name="mem_ps", bufs=2, space="PSUM"))

        # ---- weight tiles (fp8 hi/lo pairs) ----
        w_sb = {}
        for nm in ("q", "k", "v"):
            w_sb[nm] = (
                singles.tile([128, NCH, 2 * DH], F8H, tag=f"wh_{nm}", name=f"wh_{nm}"),
                singles.tile([128, NCH, 2 * DH], F8L, tag=f"wl_{nm}", name=f"wl_{nm}"),
            )

        # ---- persistent per-head state ----
        # mzb double-buffered per head: segment s reads buf[(s-1)%2] (old M)
        # while the update writes buf[s%2].
        mz_f32, mz_bf = [], []
        for hi in range(2):
            mzf = state.tile([128, DH + 1], F32, tag="mz_f32")
            bufs2 = [
                state.tile([128, DH + 1], BF16, tag="mz_bf", bufs=4,
                           name=f"mzb_{hi}_{k}")
                for k in range(2)
            ]
            mz_f32.append(mzf)
            mz_bf.append(bufs2)

        yv = y.ap().rearrange(
            "(s tile p) (h e) -> s p tile h e", p=128, tile=4, h=2
        )
        xhv = xh.ap()
        xlv = xl.ap()

        def load_slab(s):
            sh = xpool.tile([128, NCH, SEG], F8H, tag="slabh", name=f"slabh{s}")
            sl = xpool.tile([128, NCH, SEG], F8L, tag="slabl", name=f"slabl{s}")
            nc.sync.dma_start(out=sh[:], in_=xhv[:, :, ts(s, SEG)])
            nc.sync.dma_start(out=sl[:], in_=xlv[:, :, ts(s, SEG)])
            return sh, sl

        # Startup: k's projection chain is the longest (k -> elu -> transpose
        # -> uc), so its weights + slab-0 hi chunks go first, interleaved at
        # pair granularity for incremental matmul start.  Bulk / later-needed
        # loads are dispatched from cheaper DMA queues (scalar, gpsimd) to
        # dodge the ~0.6us per-dma_start SP issue cost.
        slab0h = xpool.tile([128, NCH, SEG], F8H, tag="slabh", name="slabh0")
        slab0l = xpool.tile([128, NCH, SEG], F8L, tag="slabl", name="slabl0")
        for g in range(NPAIR):
            nc.sync.dma_start(
                out=w_sb["k"][0][:, 2 * g : 2 * g + 2, :],
                in_=w_dram["k"][0].ap()[:, 2 * g : 2 * g + 2, :],
            )
            nc.sync.dma_start(
                out=slab0h[:, 2 * g : 2 * g + 2, :],
                in_=xhv[:, 2 * g : 2 * g + 2, ts(0, SEG)],
            )
        nc.sync.dma_start(out=slab0l[:], in_=xlv[:, :, ts(0, SEG)])
        nc.sync.dma_start(out=w_sb["k"][1][:], in_=w_dram["k"][1].ap())
        nc.scalar.dma_start(out=w_sb["q"][0][:], in_=w_dram["q"][0].ap())
        nc.scalar.dma_start(out=w_sb["q"][1][:], in_=w_dram["q"][1].ap())
        nc.gpsimd.dma_start(out=w_sb["v"][0][:], in_=w_dram["v"][0].ap())
        nc.gpsimd.dma_start(out=w_sb["v"][1][:], in_=w_dram["v"][1].ap())

        bias_sb = singles.tile([128, 6], F32, tag="bias")
        nc.gpsimd.dma_start(out=bias_sb[:], in_=biases.ap())
        ident = singles.tile([128, 128], BF16, tag="ident")
        nc.gpsimd.dma_start(out=ident[:], in_=ident_d.ap())
        gate_sb = singles.tile([128, 4], F32, tag="gate")
        nc.gpsimd.dma_start(out=gate_sb[:], in_=gates.ap())
        trimask = singles.tile([128, 128], BF16, tag="trimask")
        nc.gpsimd.dma_start(out=trimask[:], in_=trimask_d.ap())
        if not vbias_zero:
            vb64 = singles.tile([128, 2 * DH], BF16, tag="vb64")
            nc.gpsimd.dma_start(out=vb64[:], in_=vb64_d.ap())
            ones_inv = singles.tile([128, 128], BF16, tag="ones_inv")
            nc.gpsimd.memset(ones_inv[:], 1.0 / 128.0)

        def project_qk(s, nm, hi, slabh, slabl, dst):
            wh, wl = w_sb[nm]
            boff = 0 if nm == "q" else 2
            ps = proj_ps.tile([128, SEG], F32, tag="proj",
                              name=f"proj_{nm}_{s}_{hi}")
            hs = ts(hi, DH)
            for g in range(NPAIR):
                nc.tensor.matmul(
                    ps[:], wh[:, 2 * g : 2 * g + 2, hs],
                    slabh[:, 2 * g : 2 * g + 2, :],
                    start=(g == 0), stop=False, perf_mode=DR,
                )
            for g in range(NPAIR):
                nc.tensor.matmul(
                    ps[:], wh[:, 2 * g : 2 * g + 2, hs],
                    slabl[:, 2 * g : 2 * g + 2, :],
                    start=False, stop=False, perf_mode=DR,
                )
            for g in range(NPAIR):
                nc.tensor.matmul(
                    ps[:], wl[:, 2 * g : 2 * g + 2, hs],
                    slabh[:, 2 * g : 2 * g + 2, :],
                    start=False, stop=(g == NPAIR - 1), perf_mode=DR,
                )
            nc.scalar.activation(
                dst[:, hi, :], ps[:], AF.Identity, scale=INV_WS,
                bias=bias_sb[:, boff + hi : boff + hi + 1],
            )

        def elu1(s, src, tag, hi=None):
            """elu(x)+1 = exp(min(x,0)) + relu(x); hi=None does both heads."""
            sl = slice(None) if hi is None else slice(hi, hi + 1)
            cols = SEG if hi is not None else 2 * SEG
            mn = work.tile([128, 2, SEG], BF16, tag=f"mn_{tag}", bufs=2,
                           name=f"mn_{tag}_{s}")
            nc.vector.tensor_scalar_min(mn[:, sl, :], src[:, sl, :], 0.0)
            ex = work.tile([128, 2, SEG], BF16, tag=f"ex_{tag}", bufs=2,
                           name=f"ex_{tag}_{s}")
            nc.scalar.activation(ex[:, sl, :], mn[:, sl, :], AF.Exp)
            out = work.tile([128, 2, SEG], BF16, tag=f"s_{tag}", bufs=3,
                            name=f"s_{tag}_{s}")
            nc.vector.scalar_tensor_tensor(
                out=out[:, sl, :], in0=src[:, sl, :], scalar=0.0,
                in1=ex[:, sl, :], op0=ALU.max, op1=ALU.add,
            )
            return mn, ex, out

        def produce(s, slabh, slabl):
            """Projections, elu, layout transforms for segment s.  k first:
            its chain (k -> elu -> transpose -> uc/retr) is the longest."""
            whv, wlv = w_sb["v"]

            # ---- k projections first: per-head elu starts ASAP; the
            # elu-dependent PE transposes are emitted LAST (engine queues
            # are in-order -- an early transpose would head-of-line block
            # PE on the DVE elu chain). ----
            k_bf = work.tile([128, 2, SEG], BF16, tag="k_bf", bufs=3,
                             name=f"k_bf_{s}")
            sk_bf = None
            mn_k = ex_k = None
            for hi in range(2):
                project_qk(s, "k", hi, slabh, slabl, k_bf)
                if s < S - 1:
                    if hi == 0:
                        mn_k, ex_k, sk_bf = elu1(s, k_bf, "k", hi=0)
                    else:
                        sl = slice(1, 2)
                        nc.vector.tensor_scalar_min(
                            mn_k[:, sl, :], k_bf[:, sl, :], 0.0)
                        nc.scalar.activation(ex_k[:, sl, :], mn_k[:, sl, :], AF.Exp)
                        nc.vector.scalar_tensor_tensor(
                            out=sk_bf[:, sl, :], in0=k_bf[:, sl, :], scalar=0.0,
                            in1=ex_k[:, sl, :], op0=ALU.max, op1=ALU.add,
                        )

            # ---- q projections + batched elu ----
            q_bf = work.tile([128, 2, SEG], BF16, tag="q_bf", bufs=3,
                             name=f"q_bf_{s}")
            for hi in range(2):
                project_qk(s, "q", hi, slabh, slabl, q_bf)
            sq_bf = elu1(s, q_bf, "q")[2] if s > 0 else None

            # ---- v in natural [token, (head, dh)] layout, 3-term fp8 ----
            # v_ones [tok_p, tile, head, dh+1]: ones col feeds the softmax
            # denominator; bf16.
            v_ones = work.tile([128, 4, 2, DH + 1], BF16, tag="v_ones", bufs=3,
                               name=f"v_ones_{s}")
            nc.gpsimd.memset(v_ones[:, :, :, DH : DH + 1], 1.0)
            for half in range(2):
                psv = proj_ps.tile([128, 2, 2, DH], F32, tag="proj",
                                   name=f"projv_{s}_{half}")
                for tl in range(2):
                    tok = 2 * half + tl
                    tsl = ts(tok, 128)
                    for g in range(NPAIR):
                        nc.tensor.matmul(
                            psv[:, tl], slabh[:, 2 * g : 2 * g + 2, tsl],
                            whv[:, 2 * g : 2 * g + 2, :],
                            start=(g == 0 and tl == 0), stop=False, perf_mode=DR,
                        )
                    for g in range(NPAIR):
                        nc.tensor.matmul(
                            psv[:, tl], slabl[:, 2 * g : 2 * g + 2, tsl],
                            whv[:, 2 * g : 2 * g + 2, :],
                            start=False, stop=False, perf_mode=DR,
                        )
                    last = vbias_zero and tl == 1
                    for g in range(NPAIR):
                        nc.tensor.matmul(
                            psv[:, tl], slabh[:, 2 * g : 2 * g + 2, tsl],
                            wlv[:, 2 * g : 2 * g + 2, :],
                            start=False, stop=(last and g == NPAIR - 1),
                            perf_mode=DR,
                        )
                    if not vbias_zero:
                        nc.tensor.matmul(
                            psv[:, tl], ones_inv[:], vb64[:],
                            start=False, stop=(tl == 1),
                        )
                # psum [128, 2, 2, 128] -> v_ones[:, 2h:2h+2, :, :DH]
                nc.scalar.activation(
                    v_ones[:, 2 * half : 2 * half + 2, :, 0:DH],
                    psv[:],
                    AF.Identity, scale=INV_WS,
                )

            # ---- natural-layout sk via PE transpose (emitted last) ----
            sk_nat = None
            if s < S - 1:
                sk_nat = work.tile([128, 2, 4, DH], BF16, tag="sk_nat", bufs=3,
                                   name=f"sk_nat_{s}")
                for hi in range(2):
                    pst = proj_ps.tile([128, 4, 128], BF16, tag="proj",
                                       name=f"trp_sk_{s}_{hi}")
                    for i in range(4):
                        nc.tensor.transpose(
                            pst[:, i, :], sk_bf[:, hi, ts(i, 128)], ident[:]
                        )
                    nc.vector.tensor_copy(sk_nat[:, hi, :, :], pst[:])

            return dict(q_bf=q_bf, k_bf=k_bf, sq_bf=sq_bf, sk_bf=sk_bf,
                        v_ones=v_ones, sk_nat=sk_nat)

        for s in range(S):
            slabh, slabl = (slab0h, slab0l) if s == 0 else load_slab(s)
            pr = produce(s, slabh, slabl)
            # layout [p, tile, head, e] so the store DMA collapses to 2D
            a2_sb = outp.tile([128, 4, 2, 128], F32, tag="a2_sb", name=f"a2_{s}")
            for hi in range(2):
                _scan_phase(
                    nc, tc, s, hi, pr, gate_sb, trimask,
                    mz_f32[hi], mz_bf[hi][(s - 1) % 2], mz_bf[hi][s % 2],
                    work, small,
                    sc_ps_p, adot_ps_p, mem_ps_p,
                    a2_sb[:, :, hi, :],
                )
                if s == S - 1:
                    nc.scalar.dma_start(
                        out=yv[s, :, :, hi], in_=a2_sb[:, :, hi, :]
                    )
            if s < S - 1:
                nc.scalar.dma_start(out=yv[s], in_=a2_sb[:])


def _scan_phase(
    nc, tc, s, hi, pr, gate_sb, trimask,
    mzf, mzb_prev, mzb_new, work, small,
    sc_ps_p, adot_ps_p, mem_ps_p, a_sb,
):
    q_bf, k_bf = pr["q_bf"], pr["k_bf"]
    sq_bf, sk_bf = pr["sq_bf"], pr["sk_bf"]
    v_ones, sk_nat = pr["v_ones"], pr["sk_nat"]

    # ---------- memory state pipeline ----------
    # M update is decomposed as  M||z += sk^T @ (v||1)  +  sk^T @ (retr*(-rkn))
    # so only the second term sits on the cross-segment critical chain.
    if s < S - 1:
        uc_ps = mem_ps_p.tile([128, DH + 1], F32, tag="mem", name=f"uc_{s}_{hi}")
        for j in range(4):
            nc.tensor.matmul(
                uc_ps[:], sk_nat[:, hi, j, :], v_ones[:, j, hi, :],
                start=(j == 0), stop=(s == 0 and j == 3),
                skip_group_check=True,
            )
    # retr side (the chain): retr = sk @ M; retr_n = retr * (-rkn)
    if 0 < s < S - 1:
        retr_n = work.tile([128, 4, 128], BF16, tag="retr_n")
        for pair in range(2):
            rps = mem_ps_p.tile([128, 2, DH + 1], F32, tag="mem",
                                name=f"retr_{s}_{hi}_{pair}")
            for i2 in range(2):
                nc.tensor.matmul(
                    rps[:, i2, :], sk_bf[:, hi, ts(pair * 2 + i2, 128)],
                    mzb_prev[:],
                    start=(i2 == 0), stop=(i2 == 1), skip_group_check=True,
                )
            rkn = small.tile([128, 2], F32, tag="rkn", name=f"rkn_{s}_{hi}_{pair}")
            nc.vector.tensor_scalar(
                rkn[:], rps[:, :, DH], EPS, -1.0, ALU.add, ALU.mult
            )
            nc.vector.reciprocal(rkn[:], rkn[:])
            rkn_bc = bass.AP(
                tensor=rkn.tensor, offset=rkn.offset,
                ap=[rkn.ap[0], rkn.ap[1], [0, 128]],
            )
            nc.vector.tensor_mul(
                retr_n[:, 2 * pair : 2 * pair + 2, :], rps[:, :, :DH], rkn_bc
            )
        for j in range(4):
            nc.tensor.matmul(
                uc_ps[:, :DH], sk_nat[:, hi, j, :], retr_n[:, j, :],
                start=False, stop=(j == 3), skip_group_check=True,
            )
    if s < S - 1:
        if s == 0:
            nc.vector.tensor_copy(mzb_new[:], uc_ps[:])
            nc.vector.tensor_copy(mzf[:], uc_ps[:])
        else:
            nc.vector.scalar_tensor_tensor(
                out=mzb_new[:], in0=uc_ps[:], scalar=1.0, in1=mzf[:],
                op0=ALU.mult, op1=ALU.add,
            )
            if s < S - 2:  # mzf(S-2) has no reader (S-1 skips the update)
                nc.vector.tensor_add(mzf[:], mzf[:], uc_ps[:])

    # a_mem side (off-chain): amem_cat = gate * (sq @ M) / (sq.z + eps)
    amem_cat = None
    if s > 0:
        amem_cat = work.tile([128, 4, 128], F32, tag="amem_cat")
        for pair in range(2):
            aps = mem_ps_p.tile([128, 2, DH + 1], F32, tag="mem",
                                name=f"amem_{s}_{hi}_{pair}")
            for i2 in range(2):
                nc.tensor.matmul(
                    aps[:, i2, :], sq_bf[:, hi, ts(pair * 2 + i2, 128)],
                    mzb_prev[:],
                    start=(i2 == 0), stop=(i2 == 1), skip_group_check=True,
                )
            rg = small.tile([128, 2], F32, tag="rg", name=f"rg_{s}_{hi}_{pair}")
            nc.vector.tensor_scalar_add(rg[:], aps[:, :, DH], EPS)
            nc.vector.reciprocal(rg[:], rg[:])
            nc.vector.tensor_scalar_mul(rg[:], rg[:], gate_sb[:, 2 * hi : 2 * hi + 1])
            if s >= S - 2:
                # tail is ACT-heavy: do the scale on DVE in one bcast op
                rg_bc = bass.AP(
                    tensor=rg.tensor, offset=rg.offset,
                    ap=[rg.ap[0], rg.ap[1], [0, 128]],
                )
                nc.vector.tensor_mul(
                    amem_cat[:, 2 * pair : 2 * pair + 2, :],
                    aps[:, :, :DH], rg_bc,
                )
            else:
                for i2 in range(2):
                    nc.scalar.activation(
                        amem_cat[:, pair * 2 + i2, :], aps[:, i2, :DH],
                        AF.Identity, scale=rg[:, i2 : i2 + 1],
                    )

    # ---------- local causal attention (transposed-scores formulation) ----
    # scoresT_j [m-chunk j, t >= j*128] = k_j^T q; ACT exp writes P^T; the
    # diagonal block is masked by a Pool multiply with the 0/1 triangle;
    # a_dot accumulates against v||1 so column dh holds the denominator.
    adot_pair = []
    for pair in range(2):
        adot_pair.append(
            adot_ps_p.tile([128, 2, DH + 1], F32, tag="adot",
                           name=f"adot_{s}_{hi}_{pair}")
        )
    for j in range(4):
        t_cols = (4 - j) * 128
        sc = sc_ps_p.tile([128, SEG], F32, tag="scores")
        nc.tensor.matmul(
            sc[:, :t_cols], k_bf[:, hi, ts(j, 128)], q_bf[:, hi, j * 128 :],
            start=True, stop=True, skip_group_check=True,
        )
        ptj = work.tile([128, t_cols], BF16, tag=f"pt{j}", bufs=2,
                        name=f"pt{j}_{s}_{hi}")
        nc.scalar.activation(ptj[:], sc[:, :t_cols], AF.Exp, scale=INV_SQRT_D)
        # zero the below-diagonal entries of the diagonal block (m > t)
        nc.gpsimd.tensor_mul(ptj[:, :128], ptj[:, :128], trimask[:])
        for i in range(j, 4):
            pair, i2 = divmod(i, 2)
            # start=True clears has_written BANK-wide: only the first
            # region per bank may carry it; the second region's first
            # write stores via the already-cleared bits.
            nc.tensor.matmul(
                adot_pair[pair][:, i2, :], ptj[:, ts(i - j, 128)],
                v_ones[:, j, hi, :],
                start=(j == 0 and i2 == 0), stop=(j == i),
                skip_group_check=True,
            )

    # ---------- combine: a = a_dot * rdot + amem_cat ----------
    for pair in range(2):
        rdot = small.tile([128, 2], F32, tag="rdot", name=f"rdot_{s}_{hi}_{pair}")
        nc.vector.reciprocal(rdot[:], adot_pair[pair][:, :, DH])
        nc.vector.tensor_scalar_mul(
            rdot[:], rdot[:], gate_sb[:, 2 * hi + 1 : 2 * hi + 2]
        )
        for i2 in range(2):
            a_slice = a_sb[:, 2 * pair + i2 : 2 * pair + i2 + 1, :]
            if s > 0:
                nc.vector.scalar_tensor_tensor(
                    out=a_slice,
                    in0=adot_pair[pair][:, i2 : i2 + 1, :DH],
                    scalar=rdot[:, i2 : i2 + 1],
                    in1=amem_cat[:, 2 * pair + i2 : 2 * pair + i2 + 1, :],
                    op0=ALU.mult, op1=ALU.add,
                )
            else:
                nc.vector.tensor_scalar_mul(
                    a_slice, adot_pair[pair][:, i2 : i2 + 1, :DH],
                    rdot[:, i2 : i2 + 1],
                )


_NC_CACHE = {}


def _get_nc(vbias_zero=True):
    if vbias_zero not in _NC_CACHE:
        _NC_CACHE[vbias_zero] = _build_program(vbias_zero)
    return _NC_CACHE[vbias_zero]


def _host_consts():
    ident = np.eye(128, dtype=ml_dtypes.bfloat16)
    # trimask[m, t] = 1 iff m <= t (keep causal entries of P^T diag block)
    trimask = np.triu(np.ones((128, 128), np.float32)).astype(ml_dtypes.bfloat16)
    return ident, trimask


def _fp8_split(a):
    """a -> (e4m3 hi, e5m2 residual), both as raw fp8 arrays."""
    hi = np.asarray(a, dtype=np.float32).astype(ml_dtypes.float8_e4m3)
    lo = (a - hi.astype(np.float32)).astype(ml_dtypes.float8_e5m2)
    return hi, lo


def kernel(x, w_q, b_q, w_k, b_k, w_v, b_v, beta, _trace=False):
    global LAST_RESULTS
    x = np.asarray(x, dtype=np.float32)
    w_q = np.asarray(w_q, dtype=np.float32)
    b_q = np.asarray(b_q, dtype=np.float32)
    w_k = np.asarray(w_k, dtype=np.float32)
    b_k = np.asarray(b_k, dtype=np.float32)
    w_v = np.asarray(w_v, dtype=np.float32)
    b_v = np.asarray(b_v, dtype=np.float32)
    beta = np.asarray(beta, dtype=np.float32)

    gate = 1.0 / (1.0 + np.exp(-beta))  # sigmoid, [H]
    ident, trimask = _host_consts()
    vbias_zero = not b_v.any()

    # per-batch fp8 splits of x^T, laid out [128, chunk, T]
    xsplit = []
    for b in range(B):
        xt = np.ascontiguousarray(x[b].T).reshape(NCH, 128, T).transpose(1, 0, 2)
        xsplit.append(_fp8_split(np.ascontiguousarray(xt)))

    in_maps = []
    for c in range(8):
        b = c // 4
        h0 = (c % 4) * 2
        cols = slice(h0 * DH, (h0 + 2) * DH)
        im = {
            "xh": xsplit[b][0], "xl": xsplit[b][1],
            "ident": ident, "trimask": trimask,
        }
        for nm, w in (("q", w_q), ("k", w_k), ("v", w_v)):
            wc = np.ascontiguousarray(w[:, cols]) * WSCALE
            wc = wc.reshape(NCH, 128, 2 * DH).transpose(1, 0, 2)
            hi, lo = _fp8_split(np.ascontiguousarray(wc))
            im[f"wh{nm}"] = hi
            im[f"wl{nm}"] = lo
        bias_cols = np.stack(
            [
                b_q[h0 * DH : (h0 + 1) * DH], b_q[(h0 + 1) * DH : (h0 + 2) * DH],
                b_k[h0 * DH : (h0 + 1) * DH], b_k[(h0 + 1) * DH : (h0 + 2) * DH],
                b_v[h0 * DH : (h0 + 1) * DH], b_v[(h0 + 1) * DH : (h0 + 2) * DH],
            ],
            axis=1,
        ).astype(np.float32)  # [128, 6]
        im["biases"] = bias_cols
        g0, g1 = gate[h0], gate[h0 + 1]
        im["gates"] = np.tile(
            np.array([g0, 1.0 - g0, g1, 1.0 - g1], np.float32), (128, 1)
        )
        im["vb64"] = np.tile(
            (b_v[cols] * WSCALE).astype(ml_dtypes.bfloat16), (128, 1)
        )
        in_maps.append(im)

    nc = _get_nc(vbias_zero)
    LAST_RESULTS = bass_utils.run_bass_kernel_spmd(
        nc, in_maps, core_ids=list(range(8)), trace=_trace
    )

    out = np.empty((B, T, H * DH), np.float32)
    for c in range(8):
        b = c // 4
        h0 = (c % 4) * 2
        out[b, :, h0 * DH : (h0 + 2) * DH] = LAST_RESULTS.results[c]["out"]
    return out


# revision 32
# speedup vs baseline: 1.2756x; 1.0717x over previous
"""MultiHeadInfiniAttention Trainium2 kernel (8 NeuronCores).

Problem: B=2, T=4096, D=1024, H=8 heads x 128 dh, SEG_LEN=512 (8 segments).
Per (b,h): segment-recurrent memory (M [128,129 incl z]) + local causal
softmax attention, gated combine.

Sharding: 16 (b,h) pairs over 8 cores -> core c handles b=c//4 and heads
{2*(c%4), 2*(c%4)+1}.  Host passes per-core inputs: fp8 hi/lo splits of
x[b].T and of the weight column slices, bias/gate columns, small consts.

Projections run as scale-matched 3-term fp8 DoubleRow matmuls (4x PE rate):
  x = xh(e4m3) + xl(e5m2)           [xl at natural scale: e5m2's wide
  64*w = wh(e4m3) + wl(e5m2)         exponent range holds the residual]
  64*q = xh@wh + xl@wh + xh@wl      [all three terms share PSUM scale ->
                                     one accumulation group, no fixups]
The trailing 2^-6 rides the existing PSUM->SBUF activation copy.

v is projected directly into natural [token, dh] layout (lhsT=x chunks),
removing the PE transposes; the causal diagonal mask is applied by a Pool
(gpsimd) multiply on exp(scores) instead of a PE mask matmul.

On-device dataflow per (segment s, head h):
  - projections qT/kT [dh,512] (fp8 DoubleRow, 12 matmuls each), v_nat
    [tok,2*dh] likewise; sq/sk = elu+1 in bf16 batched across both heads
  - scoresT [m,t] (causal block-skipping); ACT exp; Pool masks the
    diagonal block; a_dot accumulates against v||1 so column dh holds the
    softmax denominator
  - memory read a_mem/retr via lhsT=sqT/skT against M||z; delta-rule
    update M||z += sk^T @ (v - retr/(sk.z) || 1)
  - combine: one scalar_tensor_tensor per 128-token chunk
    (a = a_dot*rdot + amem_cat)
"""

import os
import sys

sys.path.insert(0, os.path.dirname(os.path.abspath(__file__)))

import numpy as np
import ml_dtypes

import concourse.bass as bass
import concourse.mybir as mybir
import concourse.tile as tile
from concourse import bass_utils
from concourse.bass import ts


def split_multi_waits(nc, max_waits: int = 1) -> int:
    """This container's walrus build only supports ONE sync wait per
    instruction.  Tile emits multi-wait instructions; split the extras onto
    same-engine NOP carriers inserted right before each instruction."""
    n_split = 0
    for func in nc.m.functions:
        for bb in func.blocks:
            insts = bb.instructions
            new_list = []
            changed = False
            for inst in insts:
                si = inst.sync_info
                if si is not None and si.on_wait and len(si.on_wait) > max_waits:
                    waits = list(si.on_wait)
                    for w in waits[max_waits:]:
                        nop = mybir.InstNoOp(name=f"WSPLIT-{nc.next_id()}")
                        nop.engine = inst.engine
                        nop.sync_info = mybir.SyncInfo(on_wait=[w], on_update=[])
                        new_list.append(nop)
                        n_split += 1
                    inst.sync_info = mybir.SyncInfo(
                        on_wait=waits[:max_waits],
                        on_update=list(si.on_update or []),
                    )
                    changed = True
                new_list.append(inst)
            if changed:
                bb.instructions = new_list
    return n_split


F32 = mybir.dt.float32
BF16 = mybir.dt.bfloat16
F8H = mybir.dt.float8e4   # e4m3 (hi parts)
F8L = mybir.dt.float8e5   # e5m2 (residual parts)
AF = mybir.ActivationFunctionType
ALU = mybir.AluOpType
DR = mybir.MatmulPerfMode.DoubleRow

B, T, D = 2, 4096, 1024
H, DH, SEG = 8, 128, 512
S = T // SEG          # 8 segments
NCH = D // 128        # 8 contraction chunks (4 DoubleRow chunk-pairs)
NPAIR = NCH // 2
EPS = 1e-6
WSCALE = 64.0         # weights quantized at 64*w; 2^-6 folded into copies
INV_WS = 1.0 / WSCALE
INV_SQRT_D = 1.0 / float(np.sqrt(DH))

LAST_RESULTS = None  # BassKernelResults of the last run (for test.py)


def _build_program(vbias_zero: bool):
    nc = bass.Bass("TRN2", target_bir_lowering=False, debug=False)

    xh = nc.dram_tensor("xh", (128, NCH, T), F8H, kind="ExternalInput")
    xl = nc.dram_tensor("xl", (128, NCH, T), F8L, kind="ExternalInput")
    w_dram = {}
    for nm in ("q", "k", "v"):
        w_dram[nm] = (
            nc.dram_tensor(f"wh{nm}", (128, NCH, 2 * DH), F8H, kind="ExternalInput"),
            nc.dram_tensor(f"wl{nm}", (128, NCH, 2 * DH), F8L, kind="ExternalInput"),
        )
    biases = nc.dram_tensor("biases", (128, 6), F32, kind="ExternalInput")
    gates = nc.dram_tensor("gates", (128, 4), F32, kind="ExternalInput")
    ident_d = nc.dram_tensor("ident", (128, 128), BF16, kind="ExternalInput")
    trimask_d = nc.dram_tensor("trimask", (128, 128), BF16, kind="ExternalInput")
    vb64_d = nc.dram_tensor("vb64", (128, 2 * DH), BF16, kind="ExternalInput")
    y = nc.dram_tensor("out", (T, 2 * DH), F32, kind="ExternalOutput")

    with tile.TileContext(nc) as tc:
        _emit(nc, tc, xh, xl, w_dram, biases, gates, ident_d, trimask_d,
              vb64_d, y, vbias_zero)

    split_multi_waits(nc)
    return nc


def _emit(nc, tc, xh, xl, w_dram, biases, gates, ident_d, trimask_d,
          vb64_d, y, vbias_zero):
    from contextlib import ExitStack

    ctx = ExitStack()
    with ctx:
        singles = ctx.enter_context(tc.tile_pool(name="singles", bufs=1))
        state = ctx.enter_context(tc.tile_pool(name="state", bufs=2))
        xpool = ctx.enter_context(tc.tile_pool(name="xts", bufs=3))
        work = ctx.enter_context(tc.tile_pool(name="work", bufs=4))
        small = ctx.enter_context(tc.tile_pool(name="small", bufs=8))
        outp = ctx.enter_context(tc.tile_pool(name="outp", bufs=4))
        # PSUM pools -- exactly 8 banks
        proj_ps = ctx.enter_context(tc.tile_pool(name="proj_ps", bufs=2, space="PSUM"))
        sc_ps_p = ctx.enter_context(tc.tile_pool(name="sc_ps", bufs=2, space="PSUM"))
        adot_ps_p = ctx.enter_context(tc.tile_pool(name="adot_ps", bufs=2, space="PSUM"))
        mem_ps_p = ctx.enter_context(tc.tile_pool(name="mem_ps", bufs=2, space="PSUM"))

        # ---- weight tiles (fp8 hi/lo pairs) ----
        w_sb = {}
        for nm in ("q", "k", "v"):
            w_sb[nm] = (
                singles.tile([128, NCH, 2 * DH], F8H, tag=f"wh_{nm}", name=f"wh_{nm}"),
                singles.tile([128, NCH, 2 * DH], F8L, tag=f"wl_{nm}", name=f"wl_{nm}"),
            )

        # ---- persistent per-head state ----
        # mzb double-buffered per head: segment s reads buf[(s-1)%2] (old M)
        # while the update writes buf[s%2].
        mz_f32, mz_bf = [], []
        for hi in range(2):
            mzf = state.tile([128, DH + 1], F32, tag="mz_f32")
            bufs2 = [
                state.tile([128, DH + 1], BF16, tag="mz_bf", bufs=4,
                           name=f"mzb_{hi}_{k}")
                for k in range(2)
            ]
            mz_f32.append(mzf)
            mz_bf.append(bufs2)

        yv = y.ap().rearrange(
            "(s tile p) (h e) -> s p tile h e", p=128, tile=4, h=2
        )
        xhv = xh.ap()
        xlv = xl.ap()

        def load_slab(s):
            sh = xpool.tile([128, NCH, SEG], F8H, tag="slabh", name=f"slabh{s}")
            sl = xpool.tile([128, NCH, SEG], F8L, tag="slabl", name=f"slabl{s}")
            nc.sync.dma_start(out=sh[:], in_=xhv[:, :, ts(s, SEG)])
            nc.sync.dma_start(out=sl[:], in_=xlv[:, :, ts(s, SEG)])
            return sh, sl

        # Startup: k's projection chain is the longest (k -> elu -> transpose
        # -> uc), so its weights + slab-0 hi chunks go first, interleaved at
        # pair granularity for incremental matmul start.  Bulk / later-needed
        # loads are dispatched from cheaper DMA queues (scalar, gpsimd) to
        # dodge the ~0.6us per-dma_start SP issue cost.
        slab0h = xpool.tile([128, NCH, SEG], F8H, tag="slabh", name="slabh0")
        slab0l = xpool.tile([128, NCH, SEG], F8L, tag="slabl", name="slabl0")
        # SP queue: the critical k-hi path, interleaved in 2-pair pieces
        for g2 in range(2):
            nc.sync.dma_start(
                out=w_sb["k"][0][:, 4 * g2 : 4 * g2 + 4, :],
                in_=w_dram["k"][0].ap()[:, 4 * g2 : 4 * g2 + 4, :],
            )
            nc.sync.dma_start(
                out=slab0h[:, 4 * g2 : 4 * g2 + 4, :],
                in_=xhv[:, 4 * g2 : 4 * g2 + 4, ts(0, SEG)],
            )
        # scalar queue (in parallel): bias (needed by the first ACT copy),
        # then the lo parts of the k path
        bias_sb = singles.tile([128, 6], F32, tag="bias")
        nc.scalar.dma_start(out=bias_sb[:], in_=biases.ap())
        nc.scalar.dma_start(out=w_sb["k"][1][:], in_=w_dram["k"][1].ap())
        nc.scalar.dma_start(out=slab0l[:], in_=xlv[:, :, ts(0, SEG)])
        # SP queue continues: q weights (needed after k), then v
        nc.sync.dma_start(out=w_sb["q"][0][:], in_=w_dram["q"][0].ap())
        nc.sync.dma_start(out=w_sb["q"][1][:], in_=w_dram["q"][1].ap())
        nc.sync.dma_start(out=w_sb["v"][0][:], in_=w_dram["v"][0].ap())
        nc.sync.dma_start(out=w_sb["v"][1][:], in_=w_dram["v"][1].ap())

        ident = singles.tile([128, 128], BF16, tag="ident")
        nc.scalar.dma_start(out=ident[:], in_=ident_d.ap())
        gate_sb = singles.tile([128, 4], F32, tag="gate")
        nc.scalar.dma_start(out=gate_sb[:], in_=gates.ap())
        trimask = singles.tile([128, 128], BF16, tag="trimask")
        nc.scalar.dma_start(out=trimask[:], in_=trimask_d.ap())
        if not vbias_zero:
            vb64 = singles.tile([128, 2 * DH], BF16, tag="vb64")
            nc.scalar.dma_start(out=vb64[:], in_=vb64_d.ap())
            ones_inv = singles.tile([128, 128], BF16, tag="ones_inv")
            nc.gpsimd.memset(ones_inv[:], 1.0 / 128.0)

        def project_qk(s, nm, hi, slabh, slabl, dst):
            wh, wl = w_sb[nm]
            boff = 0 if nm == "q" else 2
            ps = proj_ps.tile([128, SEG], F32, tag="proj",
                              name=f"proj_{nm}_{s}_{hi}")
            hs = ts(hi, DH)
            # q/k run 2-term (w-residual dropped): their consumers are
            # normalized (softmax, a_mem/z), so the ~2.4% weight-quant
            # error largely cancels; v keeps all 3 terms.
            for g in range(NPAIR):
                nc.tensor.matmul(
                    ps[:], wh[:, 2 * g : 2 * g + 2, hs],
                    slabh[:, 2 * g : 2 * g + 2, :],
                    start=(g == 0), stop=False, perf_mode=DR,
                )
            for g in range(NPAIR):
                nc.tensor.matmul(
                    ps[:], wh[:, 2 * g : 2 * g + 2, hs],
                    slabl[:, 2 * g : 2 * g + 2, :],
                    start=False, stop=(g == NPAIR - 1), perf_mode=DR,
                )
            nc.scalar.activation(
                dst[:, hi, :], ps[:], AF.Identity, scale=INV_WS,
                bias=bias_sb[:, boff + hi : boff + hi + 1],
            )

        def elu1(s, src, tag, hi=None, eng=None):
            """elu(x)+1 = exp(min(x,0)) + relu(x); hi=None does both heads.
            eng picks the elementwise engine (nc.vector or nc.gpsimd) --
            the q-side chain has slack, so Pool takes it."""
            eng = eng or nc.vector
            sl = slice(None) if hi is None else slice(hi, hi + 1)
            mn = work.tile([128, 2, SEG], BF16, tag=f"mn_{tag}", bufs=2,
                           name=f"mn_{tag}_{s}")
            eng.tensor_scalar_min(mn[:, sl, :], src[:, sl, :], 0.0)
            ex = work.tile([128, 2, SEG], BF16, tag=f"ex_{tag}", bufs=2,
                           name=f"ex_{tag}_{s}")
            nc.scalar.activation(ex[:, sl, :], mn[:, sl, :], AF.Exp)
            out = work.tile([128, 2, SEG], BF16, tag=f"s_{tag}", bufs=3,
                            name=f"s_{tag}_{s}")
            eng.scalar_tensor_tensor(
                out=out[:, sl, :], in0=src[:, sl, :], scalar=0.0,
                in1=ex[:, sl, :], op0=ALU.max, op1=ALU.add,
            )
            return mn, ex, out

        def produce(s, slabh, slabl):
            """Projections, elu, layout transforms for segment s.  k first:
            its chain (k -> elu -> transpose -> uc/retr) is the longest."""
            whv, wlv = w_sb["v"]

            # ---- k projections first: per-head elu starts ASAP; the
            # elu-dependent PE transposes are emitted LAST (engine queues
            # are in-order -- an early transpose would head-of-line block
            # PE on the DVE elu chain). ----
            k_bf = work.tile([128, 2, SEG], BF16, tag="k_bf", bufs=3,
                             name=f"k_bf_{s}")
            sk_bf = None
            mn_k = ex_k = None
            for hi in range(2):
                project_qk(s, "k", hi, slabh, slabl, k_bf)
                if s < S - 1:
                    if hi == 0:
                        mn_k, ex_k, sk_bf = elu1(s, k_bf, "k", hi=0)
                    else:
                        sl = slice(1, 2)
                        nc.vector.tensor_scalar_min(
                            mn_k[:, sl, :], k_bf[:, sl, :], 0.0)
                        nc.scalar.activation(ex_k[:, sl, :], mn_k[:, sl, :], AF.Exp)
                        nc.vector.scalar_tensor_tensor(
                            out=sk_bf[:, sl, :], in0=k_bf[:, sl, :], scalar=0.0,
                            in1=ex_k[:, sl, :], op0=ALU.max, op1=ALU.add,
                        )

            # ---- q projections (elu-q deferred past v: its consumers are
            # late, and the v_ones ACT copies must not queue behind it) ----
            q_bf = work.tile([128, 2, SEG], BF16, tag="q_bf", bufs=3,
                             name=f"q_bf_{s}")
            for hi in range(2):
                project_qk(s, "q", hi, slabh, slabl, q_bf)

            # ---- v in natural [token, (head, dh)] layout, 3-term fp8 ----
            # v_ones [tok_p, tile, head, dh+1]: the extra column is -1 so
            # every accumulated denominator (z, sq.z, sk.z, softmax sum)
            # comes out NEGATED -- retr_n is then a single tensor divide,
            # and the gate columns carry the compensating minus sign.
            v_ones = work.tile([128, 4, 2, DH + 1], BF16, tag="v_ones", bufs=3,
                               name=f"v_ones_{s}")
            nc.gpsimd.memset(v_ones[:, :, :, DH : DH + 1], -1.0)
            for half in range(2):
                psv = proj_ps.tile([128, 2, 2, DH], F32, tag="proj",
                                   name=f"projv_{s}_{half}")
                for tl in range(2):
                    tok = 2 * half + tl
                    tsl = ts(tok, 128)
                    for g in range(NPAIR):
                        nc.tensor.matmul(
                            psv[:, tl], slabh[:, 2 * g : 2 * g + 2, tsl],
                            whv[:, 2 * g : 2 * g + 2, :],
                            start=(g == 0 and tl == 0), stop=False, perf_mode=DR,
                        )
                    for g in range(NPAIR):
                        nc.tensor.matmul(
                            psv[:, tl], slabl[:, 2 * g : 2 * g + 2, tsl],
                            whv[:, 2 * g : 2 * g + 2, :],
                            start=False, stop=False, perf_mode=DR,
                        )
                    last = vbias_zero and tl == 1
                    for g in range(NPAIR):
                        nc.tensor.matmul(
                            psv[:, tl], slabh[:, 2 * g : 2 * g + 2, tsl],
                            wlv[:, 2 * g : 2 * g + 2, :],
                            start=False, stop=(last and g == NPAIR - 1),
                            perf_mode=DR,
                        )
                    if not vbias_zero:
                        nc.tensor.matmul(
                            psv[:, tl], ones_inv[:], vb64[:],
                            start=False, stop=(tl == 1),
                        )
                # psum [128, 2, 2, 128] -> v_ones[:, 2h:2h+2, :, :DH]
                nc.scalar.activation(
                    v_ones[:, 2 * half : 2 * half + 2, :, 0:DH],
                    psv[:],
                    AF.Identity, scale=INV_WS,
                )

            sq_bf = elu1(s, q_bf, "q")[2] if s > 0 else None

            # ---- natural-layout sk via PE transpose (emitted last) ----
            sk_nat = None
            if s < S - 1:
                sk_nat = work.tile([128, 2, 4, DH], BF16, tag="sk_nat", bufs=3,
                                   name=f"sk_nat_{s}")
                for hi in range(2):
                    pst = proj_ps.tile([128, 4, 128], BF16, tag="proj",
                                       name=f"trp_sk_{s}_{hi}")
                    for i in range(4):
                        nc.tensor.transpose(
                            pst[:, i, :], sk_bf[:, hi, ts(i, 128)], ident[:]
                        )
                    nc.vector.tensor_copy(sk_nat[:, hi, :, :], pst[:])

            return dict(q_bf=q_bf, k_bf=k_bf, sq_bf=sq_bf, sk_bf=sk_bf,
                        v_ones=v_ones, sk_nat=sk_nat)

        for s in range(S):
            slabh, slabl = (slab0h, slab0l) if s == 0 else load_slab(s)
            pr = produce(s, slabh, slabl)
            # layout [p, tile, head, e] so the store DMA collapses to 2D
            a2_sb = outp.tile([128, 4, 2, 128], F32, tag="a2_sb", name=f"a2_{s}")
            for hi in range(2):
                _scan_phase(
                    nc, tc, s, hi, pr, gate_sb, trimask,
                    mz_f32[hi], mz_bf[hi][(s - 1) % 2], mz_bf[hi][s % 2],
                    work, small,
                    sc_ps_p, adot_ps_p, mem_ps_p,
                    a2_sb[:, :, hi, :],
                )
                if s == S - 1:
                    nc.scalar.dma_start(
                        out=yv[s, :, :, hi], in_=a2_sb[:, :, hi, :]
                    )
            if s < S - 1:
                nc.scalar.dma_start(out=yv[s], in_=a2_sb[:])


def _scan_phase(
    nc, tc, s, hi, pr, gate_sb, trimask,
    mzf, mzb_prev, mzb_new, work, small,
    sc_ps_p, adot_ps_p, mem_ps_p, a_sb,
):
    q_bf, k_bf = pr["q_bf"], pr["k_bf"]
    sq_bf, sk_bf = pr["sq_bf"], pr["sk_bf"]
    v_ones, sk_nat = pr["v_ones"], pr["sk_nat"]

    # ---------- local causal attention (chain-free: emitted first) ----
    # scoresT_j [m-chunk j, t >= j*128] = k_j^T q; ACT exp writes P^T; the
    # diagonal block is masked by a Pool multiply with the 0/1 triangle;
    # a_dot accumulates against v||1 so column dh holds the denominator.
    # The diagonal a_dot matmul of each j-group goes LAST so the Pool mask
    # latency hides behind the off-diagonal matmuls.
    adot_pair = []
    for pair in range(2):
        adot_pair.append(
            adot_ps_p.tile([128, 2, DH + 1], F32, tag="adot",
                           name=f"adot_{s}_{hi}_{pair}")
        )
    for j in range(4):
        t_cols = (4 - j) * 128
        sc = sc_ps_p.tile([128, SEG], F32, tag="scores")
        nc.tensor.matmul(
            sc[:, :t_cols], k_bf[:, hi, ts(j, 128)], q_bf[:, hi, j * 128 :],
            start=True, stop=True, skip_group_check=True,
        )
        ptj = work.tile([128, t_cols], BF16, tag=f"pt{j}", bufs=2,
                        name=f"pt{j}_{s}_{hi}")
        nc.scalar.activation(ptj[:], sc[:, :t_cols], AF.Exp, scale=INV_SQRT_D)
        # zero the below-diagonal entries of the diagonal block (m > t)
        nc.gpsimd.tensor_mul(ptj[:, :128], ptj[:, :128], trimask[:])
        for i in list(range(j + 1, 4)) + [j]:
            pair, i2 = divmod(i, 2)
            # start=True clears has_written BANK-wide: it must ride the
            # first-EMITTED write to each bank (i=1 for pair 0 under the
            # diag-last order, i=2 for pair 1); every other region write
            # stores via the already-cleared bits.
            nc.tensor.matmul(
                adot_pair[pair][:, i2, :], ptj[:, ts(i - j, 128)],
                v_ones[:, j, hi, :],
                start=(j == 0 and i in (1, 2)), stop=(j == i),
                skip_group_check=True,
            )

    # ---------- memory state pipeline ----------
    # M update is decomposed as  M||z += sk^T @ (v||1)  +  sk^T @ (retr*(-rkn))
    # so only the second term sits on the cross-segment critical chain.
    # PE order: uc part 1, retr, amem (covers the rkn/retr_n DVE latency),
    # then uc part 2.
    if s < S - 1:
        uc_ps = mem_ps_p.tile([128, DH + 1], F32, tag="mem", name=f"uc_{s}_{hi}")
        for j in range(4):
            nc.tensor.matmul(
                uc_ps[:], sk_nat[:, hi, j, :], v_ones[:, j, hi, :],
                start=(j == 0), stop=(s == 0 and j == 3),
                skip_group_check=True,
            )
    # retr side (the chain): retr = sk @ M; retr_n = retr * (-rkn)
    retr_n = None
    if 0 < s < S - 1:
        retr_n = work.tile([128, 4, 128], BF16, tag="retr_n")
        for pair in range(2):
            rps = mem_ps_p.tile([128, 2, DH + 1], F32, tag="mem",
                                name=f"retr_{s}_{hi}_{pair}")
            for i2 in range(2):
                nc.tensor.matmul(
                    rps[:, i2, :], sk_bf[:, hi, ts(pair * 2 + i2, 128)],
                    mzb_prev[:],
                    start=(i2 == 0), stop=(i2 == 1), skip_group_check=True,
                )
            # retr_n = retr * recip(-den): den column is already negated
            # and den = sk.z >> eps at s >= 1, so eps is dropped and the
            # old add/negate ops vanish.
            rkn = small.tile([128, 2], F32, tag="rkn", name=f"rkn_{s}_{hi}_{pair}")
            nc.vector.reciprocal(rkn[:], rps[:, :, DH])
            rkn_bc = bass.AP(
                tensor=rkn.tensor, offset=rkn.offset,
                ap=[rkn.ap[0], rkn.ap[1], [0, 128]],
            )
            nc.vector.tensor_mul(
                retr_n[:, 2 * pair : 2 * pair + 2, :], rps[:, :, :DH], rkn_bc
            )

    # a_mem side (off-chain): amem_cat = gate * (sq @ M) / (sq.z + eps)
    amem_cat = None
    if s > 0:
        amem_cat = work.tile([128, 4, 128], F32, tag="amem_cat")
        for pair in range(2):
            aps = mem_ps_p.tile([128, 2, DH + 1], F32, tag="mem",
                                name=f"amem_{s}_{hi}_{pair}")
            for i2 in range(2):
                nc.tensor.matmul(
                    aps[:, i2, :], sq_bf[:, hi, ts(pair * 2 + i2, 128)],
                    mzb_prev[:],
                    start=(i2 == 0), stop=(i2 == 1), skip_group_check=True,
                )
            rg = small.tile([128, 2], F32, tag="rg", name=f"rg_{s}_{hi}_{pair}")
            nc.vector.tensor_scalar_add(rg[:], aps[:, :, DH], -EPS)
            nc.vector.reciprocal(rg[:], rg[:])
            nc.vector.tensor_scalar_mul(rg[:], rg[:], gate_sb[:, 2 * hi : 2 * hi + 1])
            if s >= S - 2:
                # tail is ACT-heavy: do the scale on DVE in one bcast op
                rg_bc = bass.AP(
                    tensor=rg.tensor, offset=rg.offset,
                    ap=[rg.ap[0], rg.ap[1], [0, 128]],
                )
                nc.vector.tensor_mul(
                    amem_cat[:, 2 * pair : 2 * pair + 2, :],
                    aps[:, :, :DH], rg_bc,
                )
            else:
                for i2 in range(2):
                    nc.scalar.activation(
                        amem_cat[:, pair * 2 + i2, :], aps[:, i2, :DH],
                        AF.Identity, scale=rg[:, i2 : i2 + 1],
                    )

    # uc part 2 + state roll-over
    if retr_n is not None:
        for j in range(4):
            nc.tensor.matmul(
                uc_ps[:, :DH], sk_nat[:, hi, j, :], retr_n[:, j, :],
                start=False, stop=(j == 3), skip_group_check=True,
            )
    if s < S - 1:
        if s == 0:
            nc.vector.tensor_copy(mzb_new[:], uc_ps[:])
            nc.vector.tensor_copy(mzf[:], uc_ps[:])
        else:
            nc.vector.scalar_tensor_tensor(
                out=mzb_new[:], in0=uc_ps[:], scalar=1.0, in1=mzf[:],
                op0=ALU.mult, op1=ALU.add,
            )
            if s < S - 2:  # mzf(S-2) has no reader (S-1 skips the update)
                nc.vector.tensor_add(mzf[:], mzf[:], uc_ps[:])

    # ---------- combine: a = a_dot * rdot + amem_cat ----------
    for pair in range(2):
        rdot = small.tile([128, 2], F32, tag="rdot", name=f"rdot_{s}_{hi}_{pair}")
        nc.vector.reciprocal(rdot[:], adot_pair[pair][:, :, DH])
        nc.vector.tensor_scalar_mul(
            rdot[:], rdot[:], gate_sb[:, 2 * hi + 1 : 2 * hi + 2]
        )
        for i2 in range(2):
            a_slice = a_sb[:, 2 * pair + i2 : 2 * pair + i2 + 1, :]
            if s > 0:
                nc.vector.scalar_tensor_tensor(
                    out=a_slice,
                    in0=adot_pair[pair][:, i2 : i2 + 1, :DH],
                    scalar=rdot[:, i2 : i2 + 1],
                    in1=amem_cat[:, 2 * pair + i2 : 2 * pair + i2 + 1, :],
                    op0=ALU.mult, op1=ALU.add,
                )
            else:
                nc.vector.tensor_scalar_mul(
                    a_slice, adot_pair[pair][:, i2 : i2 + 1, :DH],
                    rdot[:, i2 : i2 + 1],
                )


_NC_CACHE = {}


def _get_nc(vbias_zero=True):
    if vbias_zero not in _NC_CACHE:
        _NC_CACHE[vbias_zero] = _build_program(vbias_zero)
    return _NC_CACHE[vbias_zero]


def _host_consts():
    ident = np.eye(128, dtype=ml_dtypes.bfloat16)
    # trimask[m, t] = 1 iff m <= t (keep causal entries of P^T diag block)
    trimask = np.triu(np.ones((128, 128), np.float32)).astype(ml_dtypes.bfloat16)
    return ident, trimask


def _fp8_split(a):
    """a -> (e4m3 hi, e5m2 residual), both as raw fp8 arrays."""
    hi = np.asarray(a, dtype=np.float32).astype(ml_dtypes.float8_e4m3)
    lo = (a - hi.astype(np.float32)).astype(ml_dtypes.float8_e5m2)
    return hi, lo


def kernel(x, w_q, b_q, w_k, b_k, w_v, b_v, beta, _trace=False):
    global LAST_RESULTS
    x = np.asarray(x, dtype=np.float32)
    w_q = np.asarray(w_q, dtype=np.float32)
    b_q = np.asarray(b_q, dtype=np.float32)
    w_k = np.asarray(w_k, dtype=np.float32)
    b_k = np.asarray(b_k, dtype=np.float32)
    w_v = np.asarray(w_v, dtype=np.float32)
    b_v = np.asarray(b_v, dtype=np.float32)
    beta = np.asarray(beta, dtype=np.float32)

    gate = 1.0 / (1.0 + np.exp(-beta))  # sigmoid, [H]
    ident, trimask = _host_consts()
    vbias_zero = not b_v.any()

    # per-batch fp8 splits of x^T, laid out [128, chunk, T]
    xsplit = []
    for b in range(B):
        xt = np.ascontiguousarray(x[b].T).reshape(NCH, 128, T).transpose(1, 0, 2)
        xsplit.append(_fp8_split(np.ascontiguousarray(xt)))

    in_maps = []
    for c in range(8):
        b = c // 4
        h0 = (c % 4) * 2
        cols = slice(h0 * DH, (h0 + 2) * DH)
        im = {
            "xh": xsplit[b][0], "xl": xsplit[b][1],
            "ident": ident, "trimask": trimask,
        }
        for nm, w in (("q", w_q), ("k", w_k), ("v", w_v)):
            wc = np.ascontiguousarray(w[:, cols]) * WSCALE
            wc = wc.reshape(NCH, 128, 2 * DH).transpose(1, 0, 2)
            hi, lo = _fp8_split(np.ascontiguousarray(wc))
            im[f"wh{nm}"] = hi
            im[f"wl{nm}"] = lo
        bias_cols = np.stack(
            [
                b_q[h0 * DH : (h0 + 1) * DH], b_q[(h0 + 1) * DH : (h0 + 2) * DH],
                b_k[h0 * DH : (h0 + 1) * DH], b_k[(h0 + 1) * DH : (h0 + 2) * DH],
                b_v[h0 * DH : (h0 + 1) * DH], b_v[(h0 + 1) * DH : (h0 + 2) * DH],
            ],
            axis=1,
        ).astype(np.float32)  # [128, 6]
        im["biases"] = bias_cols
        # negated: on-device denominators are accumulated with a -1 column,
        # so the gate factors carry the compensating sign
        g0, g1 = gate[h0], gate[h0 + 1]
        im["gates"] = np.tile(
            np.array([-g0, -(1.0 - g0), -g1, -(1.0 - g1)], np.float32), (128, 1)
        )
        im["vb64"] = np.tile(
            (b_v[cols] * WSCALE).astype(ml_dtypes.bfloat16), (128, 1)
        )
        in_maps.append(im)

    nc = _get_nc(vbias_zero)
    LAST_RESULTS = bass_utils.run_bass_kernel_spmd(
        nc, in_maps, core_ids=list(range(8)), trace=_trace
    )

    out = np.empty((B, T, H * DH), np.float32)
    for c in range(8):
        b = c // 4
        h0 = (c % 4) * 2
        out[b, :, h0 * DH : (h0 + 2) * DH] = LAST_RESULTS.results[c]["out"]
    return out
